# revision 8
# baseline (speedup 1.0000x reference)
"""Trainium2 kernel for AttentionContextExtractor.

reference semantics (B=4, L=2048, D=1024, H=16, HD=64):
    q = (x @ Wq.T)  [B,L,D] -> heads
    scores = q k^T / sqrt(HD), causal mask, softmax
    context = softmax(scores) v   (concat heads)  -> [B,L,D]
    output = context @ Wo.T
    returns (output, context)

Sharding over 8 NeuronCores: 2-way data-parallel over batch x 4-way
tensor-parallel over heads. Each core computes 4 heads for 2 batches:
q/k/v projections for its 256 channels, causal flash-attention in a
transposed layout (S^T[k,q]) and a partial o-projection. Host sums the
4 o-projection partials per batch group and concatenates context
channels. No collectives.

All matmuls run in float32r (1 cycle/row on the PE when N>=256,
~1e-4 relative error), accumulating in fp32 PSUM. The softmax
denominator is fused into the PV matmul as a 65th lhsT column of ones.
"""
import functools
import os
import numpy as np

LAST_RESULTS = None

B, L, D = 4, 2048, 1024
H, HD = 16, 64
NCORES, DP, TP = 8, 2, 4
B_LOC = B // DP            # 2 batches per core
C_LOC = D // TP            # 256 channels (4 heads) per core
T_LOC = B_LOC * L          # 4096 tokens per core
P = 128                    # partitions
QCH = 512                  # q-chunk
NQC = L // QCH             # 4 q-chunks per batch
NKT = L // P               # 16 k-tiles per batch
DT = 8                     # 1024/128 contraction tiles for projections


def _build(debug=False):
    import concourse.bacc as bacc
    import concourse.tile as tile
    from concourse import mybir
    from concourse.masks import make_lower_triangular, make_identity

    F32 = mybir.dt.float32
    F32R = mybir.dt.float32r
    EXP = mybir.ActivationFunctionType.Exp
    IDENT = mybir.ActivationFunctionType.Identity

    nc = bacc.Bacc("TRN2", target_bir_lowering=False)

    xT = nc.dram_tensor("xT", [D, T_LOC], F32R, kind="ExternalInput")
    wqT = nc.dram_tensor("wqT", [D, C_LOC], F32R, kind="ExternalInput")
    wkT = nc.dram_tensor("wkT", [D, C_LOC], F32R, kind="ExternalInput")
    wvT = nc.dram_tensor("wvT", [D, C_LOC], F32R, kind="ExternalInput")
    woT = nc.dram_tensor("woT", [C_LOC, D], F32R, kind="ExternalInput")
    out_part = nc.dram_tensor("out_part", [T_LOC, D], F32, kind="ExternalOutput")
    ctx_part = nc.dram_tensor("ctx_part", [T_LOC, C_LOC], F32, kind="ExternalOutput")
    if debug:
        dbg_tri = nc.dram_tensor("dbg_tri", [P, P], F32, kind="ExternalOutput")
        dbg_st = nc.dram_tensor("dbg_st", [P, QCH], F32, kind="ExternalOutput")
        dbg_p = nc.dram_tensor("dbg_p", [P, QCH], F32R, kind="ExternalOutput")
        dbg_ot = nc.dram_tensor("dbg_ot", [P, QCH], F32, kind="ExternalOutput")
        dbg_v = nc.dram_tensor("dbg_v", [P, HD + 1], F32R, kind="ExternalOutput")
        dbg_qt = nc.dram_tensor("dbg_qt", [P, QCH], F32R, kind="ExternalOutput")
        dbg_kt = nc.dram_tensor("dbg_kt", [P, P], F32R, kind="ExternalOutput")
        dbg_rbc = nc.dram_tensor("dbg_rbc", [P, QCH], F32, kind="ExternalOutput")

    xT_r = xT.rearrange("(t p) n -> p t n", p=P)      # [128, 8, 4096]
    wqT_r = wqT.rearrange("(t p) c -> p t c", p=P)    # [128, 8, 256]
    wkT_r = wkT.rearrange("(t p) c -> p t c", p=P)
    wvT_r = wvT.rearrange("(t p) c -> p t c", p=P)
    woT_r = woT.rearrange("(ct p) o -> p ct o", p=P)  # [128, 2, 1024]
    outp_r = out_part.rearrange("(t p) o -> p t o", p=P)   # [128, 32, 1024]
    ctxp_r = ctx_part.rearrange("(t p) c -> p t c", p=P)   # [128, 32, 256]

    with tile.TileContext(nc) as tc:
        with (
            tc.tile_pool(name="consts", bufs=1) as consts,
            tc.tile_pool(name="xp", bufs=2) as xp,
            tc.tile_pool(name="qk", bufs=1) as qkpool,
            tc.tile_pool(name="vp", bufs=1) as vpool,
            tc.tile_pool(name="pp", bufs=8) as ppool,
            tc.tile_pool(name="cx", bufs=4) as cxpool,
            tc.tile_pool(name="ep", bufs=2) as epool,
            tc.tile_pool(name="ob", bufs=2) as obpool,
            tc.tile_pool(name="psA", bufs=4, space="PSUM") as psA,
            tc.tile_pool(name="psB", bufs=2, space="PSUM") as psB,
            tc.tile_pool(name="psC", bufs=2, space="PSUM") as psC,
        ):
            # ---- constants ----
            wq_sb = consts.tile([P, DT, C_LOC], F32R, tag="wq")
            wk_sb = consts.tile([P, DT, C_LOC], F32R, tag="wk")
            wv_sb = consts.tile([P, DT, C_LOC], F32R, tag="wv")
            wo_sb = consts.tile([P, 2, D], F32R, tag="wo")
            nc.sync.dma_start(wq_sb[:], wqT_r)
            nc.sync.dma_start(wk_sb[:], wkT_r)
            nc.sync.dma_start(wv_sb[:], wvT_r)
            nc.sync.dma_start(wo_sb[:], woT_r)
            tri = consts.tile([P, P], F32, tag="tri")
            make_lower_triangular(nc, tri[:], val=-1e9, diag=False)
            ident = consts.tile([P, P], F32, tag="ident")
            make_identity(nc, ident[:])

            # V layout: [128k, kt, head, 64 V-chans + 1 ones]  (fused l-sum)
            v_sb = vpool.tile([P, NKT, 4, HD + 1], F32R, tag="v")
            nc.scalar.activation(
                v_sb[:, :, :, HD],
                tri[:, 0:NKT * 4].rearrange("p (a b) -> p a b", a=NKT),
                IDENT, bias=1.0, scale=0.0,
            )

            qt_sb = qkpool.tile([P, 2, L], F32R, tag="qt")
            kt_sb = qkpool.tile([P, 2, L], F32R, tag="kt")

            def emit_pv(job, pair, nk):
                kt, j0, p_h, ot = job
                for h in range(2):
                    nc.tensor.matmul(
                        ot[h][0:HD + 1, j0:QCH],
                        v_sb[:, kt, pair * 2 + h, :],
                        p_h[h][:, j0:QCH],
                        start=(kt == 0), stop=(kt == nk - 1),
                    )

            for b in range(B_LOC):
                # ---- phase 1: projections for batch b ----
                for c in range(NQC):
                    xc = xp.tile([P, DT, QCH], F32R, tag="xc")
                    nc.sync.dma_start(
                        xc[:], xT_r[:, :, b * L + c * QCH:b * L + (c + 1) * QCH]
                    )
                    for w_sb, dst in ((wq_sb, qt_sb), (wk_sb, kt_sb)):
                        for ct in range(2):
                            ps = psA.tile([P, QCH], F32, tag="psA")
                            for d in range(DT):
                                nc.tensor.matmul(
                                    ps[:], w_sb[:, d, ct * P:(ct + 1) * P], xc[:, d, :],
                                    start=(d == 0), stop=(d == DT - 1),
                                )
                            nc.vector.tensor_copy(dst[:, ct, c * QCH:(c + 1) * QCH], ps[:])
                    for tt in range(4):
                        ps = psC.tile([P, QCH], F32, tag="psC")
                        for d in range(DT):
                            nc.tensor.matmul(
                                ps[:, 0:C_LOC], xc[:, d, tt * P:(tt + 1) * P], wv_sb[:, d, :],
                                start=(d == 0), stop=(d == DT - 1),
                            )
                        nc.vector.tensor_copy(
                            v_sb[:, c * 4 + tt, :, 0:HD],
                            ps[:, 0:C_LOC].rearrange("p (h e) -> p h e", h=4),
                        )

                # ---- phase 2: attention for batch b ----
                for qc in range(NQC):
                    nk = 4 * qc + 4
                    ctx_tiles = []
                    for pair in range(2):
                        ot = [
                            psB.tile([P, QCH], F32, tag="psB", name=f"ot_{b}_{qc}_{pair}_{h}")
                            for h in range(2)
                        ]
                        pending = []
                        for kt in range(nk):
                            m = kt - 4 * qc
                            j0 = max(0, m * P)
                            p_h = []
                            for h in range(2):
                                st = psA.tile([P, QCH], F32, tag="psA")
                                nc.tensor.matmul(
                                    st[:, j0:QCH],
                                    kt_sb[h * HD:(h + 1) * HD, pair, kt * P:(kt + 1) * P],
                                    qt_sb[h * HD:(h + 1) * HD, pair,
                                          qc * QCH + j0:(qc + 1) * QCH],
                                    start=True, stop=True,
                                    tile_position=(h * HD, 0),
                                )
                                if m >= 0:
                                    nc.vector.tensor_add(
                                        st[:, j0:j0 + P], st[:, j0:j0 + P], tri[:]
                                    )
                                pt = ppool.tile([P, QCH], F32R, tag="p")
                                nc.scalar.activation(
                                    pt[:, j0:QCH], st[:, j0:QCH], EXP, scale=0.125
                                )
                                if debug and b == 0 and qc == 0 and pair == 0 and kt == 0 and h == 0:
                                    dbg_sb = consts.tile([P, QCH], F32, tag="dbg_sb")
                                    nc.vector.tensor_copy(dbg_sb[:], st[:])
                                    nc.sync.dma_start(dbg_st[:], dbg_sb[:])
                                    nc.sync.dma_start(dbg_p[:], pt[:])
                                    nc.sync.dma_start(dbg_tri[:], tri[:])
                                    nc.sync.dma_start(dbg_v[:], v_sb[:, 0, 0, :])
                                    nc.sync.dma_start(dbg_qt[:], qt_sb[:, 0, 0:QCH])
                                    nc.sync.dma_start(dbg_kt[:], kt_sb[:, 0, 0:P])
                                p_h.append(pt)
                            pending.append((kt, j0, p_h, ot))
                            if len(pending) > 2:
                                emit_pv(pending.pop(0), pair, nk)
                        for job in pending:
                            emit_pv(job, pair, nk)

                        # epilogue: normalize -> ctxT [128c, 512q] for this pair
                        ctxT = cxpool.tile([P, QCH], F32R, tag="ctxT")
                        for h in range(2):
                            lsb = epool.tile([P, QCH], F32, tag="lsb")
                            nc.vector.reciprocal(lsb[HD:HD + 1, :], ot[h][HD:HD + 1, :])
                            rec = epool.tile([1, QCH], F32, tag="rec")
                            nc.sync.dma_start(rec[:], lsb[HD:HD + 1, :])
                            rbc = epool.tile([P, QCH], F32, tag="rbc")
                            nc.gpsimd.partition_broadcast(
                                rbc[0:HD, :], rec[:], channels=HD
                            )
                            if debug and b == 0 and qc == 0 and pair == 0 and h == 0:
                                dbg_sb2 = consts.tile([P, QCH], F32, tag="dbg_sb2")
                                nc.vector.tensor_copy(dbg_sb2[:], ot[h][:])
                                nc.sync.dma_start(dbg_ot[:], dbg_sb2[:])
                                nc.sync.dma_start(dbg_rbc[:], rbc[:])
                            if h == 0:
                                nc.vector.tensor_mul(
                                    ctxT[0:HD, :], ot[h][0:HD, :], rbc[0:HD, :]
                                )
                            else:
                                tmp = epool.tile([HD, QCH], F32R, tag="tmp")
                                nc.vector.tensor_mul(tmp[:], ot[h][0:HD, :], rbc[0:HD, :])
                                nc.sync.dma_start(ctxT[HD:P, :], tmp[:])
                        ctx_tiles.append(ctxT)

                    # ---- context transposes (pair 0 covers pair-1 epilogue latency) ----
                    csb = obpool.tile([P, 4, 2, P], F32, tag="csb")
                    for qt in range(4):
                        tps = psC.tile([P, QCH], F32, tag="psC")
                        nc.tensor.transpose(
                            tps[:, 0:P],
                            ctx_tiles[0][:, qt * P:(qt + 1) * P].bitcast(F32),
                            ident[:],
                        )
                        nc.vector.tensor_copy(csb[:, qt, 0, :], tps[:, 0:P])
                    # ---- o-projection for this q-chunk ----
                    for qt in range(4):
                        row = b * NKT + qc * 4 + qt
                        osb = obpool.tile([P, D], F32, tag="osb")
                        for oc in range(2):
                            ops = psA.tile([P, QCH], F32, tag="psA")
                            for ct in range(2):
                                nc.tensor.matmul(
                                    ops[:], ctx_tiles[ct][:, qt * P:(qt + 1) * P],
                                    wo_sb[:, ct, oc * QCH:(oc + 1) * QCH],
                                    start=(ct == 0), stop=(ct == 1),
                                )
                            nc.vector.tensor_copy(osb[:, oc * QCH:(oc + 1) * QCH], ops[:])
                        nc.sync.dma_start(outp_r[:, row, :], osb[:])
                    for qt in range(4):
                        row = b * NKT + qc * 4 + qt
                        tps = psC.tile([P, QCH], F32, tag="psC")
                        nc.tensor.transpose(
                            tps[:, 0:P],
                            ctx_tiles[1][:, qt * P:(qt + 1) * P].bitcast(F32),
                            ident[:],
                        )
                        nc.vector.tensor_copy(csb[:, qt, 1, :], tps[:, 0:P])
                        nc.sync.dma_start(ctxp_r[:, row, :], csb[:, qt, :, :])

    nc.compile()
    return nc


@functools.cache
def _get_nc():
    return _build()


def _numpy_fallback(x, Wq, Wk, Wv, Wo, attn_mask):
    Bx, Lx, Dx = x.shape
    hd = D // H
    nh = Dx // hd
    scale = hd ** -0.5
    xf = x.astype(np.float64)
    q = (xf @ Wq.T.astype(np.float64)).reshape(Bx, Lx, nh, hd).transpose(0, 2, 1, 3)
    k = (xf @ Wk.T.astype(np.float64)).reshape(Bx, Lx, nh, hd).transpose(0, 2, 1, 3)
    v = (xf @ Wv.T.astype(np.float64)).reshape(Bx, Lx, nh, hd).transpose(0, 2, 1, 3)
    out = np.empty((Bx, Lx, Dx), np.float64)
    ctx = np.empty((Bx, Lx, Dx), np.float64)
    mask = attn_mask[0] if attn_mask.ndim == 3 else attn_mask
    for bi in range(Bx):
        heads = []
        for hi in range(nh):
            s = q[bi, hi] @ k[bi, hi].T * scale
            s = np.where(mask == 0, -np.inf, s)
            s -= s.max(axis=-1, keepdims=True)
            p = np.exp(s)
            p /= p.sum(axis=-1, keepdims=True)
            heads.append(p @ v[bi, hi])
        c = np.stack(heads, axis=0).transpose(1, 0, 2).reshape(Lx, Dx)
        ctx[bi] = c
        out[bi] = c @ Wo.T.astype(np.float64)
    return out.astype(np.float32), ctx.astype(np.float32)


def kernel(x, Wq, Wk, Wv, Wo, attn_mask):
    x = np.asarray(x)
    Wq, Wk, Wv, Wo = (np.asarray(a) for a in (Wq, Wk, Wv, Wo))
    attn_mask = np.asarray(attn_mask)

    causal = (
        x.shape == (B, L, D)
        and attn_mask.shape == (1, L, L)
        and bool(np.array_equal(attn_mask[0], np.tril(np.ones((L, L), attn_mask.dtype))))
    )
    if not causal:
        return _numpy_fallback(x, Wq, Wk, Wv, Wo, attn_mask)

    from concourse.bass_utils import run_bass_kernel_spmd

    nc = _get_nc()
    in_maps = []
    for g in range(DP):
        xTg = np.ascontiguousarray(
            x[g * B_LOC:(g + 1) * B_LOC].reshape(T_LOC, D).T
        ).astype(np.float32)
        for t in range(TP):
            rows = slice(t * C_LOC, (t + 1) * C_LOC)
            in_maps.append({
                "xT": xTg,
                "wqT": np.ascontiguousarray(Wq[rows].T).astype(np.float32),
                "wkT": np.ascontiguousarray(Wk[rows].T).astype(np.float32),
                "wvT": np.ascontiguousarray(Wv[rows].T).astype(np.float32),
                "woT": np.ascontiguousarray(Wo[:, rows].T).astype(np.float32),
            })

    global LAST_RESULTS
    LAST_RESULTS = run_bass_kernel_spmd(nc, in_maps, core_ids=list(range(NCORES)))
    res = LAST_RESULTS.results

    output = np.empty((B, L, D), np.float32)
    context = np.empty((B, L, D), np.float32)
    for g in range(DP):
        acc = res[g * TP]["out_part"].astype(np.float32).copy()
        for t in range(1, TP):
            acc += res[g * TP + t]["out_part"]
        output[g * B_LOC:(g + 1) * B_LOC] = acc.reshape(B_LOC, L, D)
        for t in range(TP):
            context[g * B_LOC:(g + 1) * B_LOC, :, t * C_LOC:(t + 1) * C_LOC] = (
                res[g * TP + t]["ctx_part"].reshape(B_LOC, L, C_LOC)
            )
    return output, context
